# revision 8
# baseline (speedup 1.0000x reference)
"""Trainium2 Bass kernel for nn_Contrastive_Loss (bs=8192, hidden=2048, 8 cores).

Math: reference(X, Y) = cl(X,Y) + cl(Y,X) where
  cl(A,B)[i] = -log(E_ii / (colsum_i(E) - E_ii)),  E = exp(norm(A) @ norm(B).T)
Since norm(Y)@norm(X).T = S.T, the second term's column sums are the first
term's row sums and the diagonals coincide.  With E = exp(S):
  out[i] = log(rowsum_i(E) - E_ii) + log(colsum_i(E) - E_ii) - 2*S_ii

Layout: the S block is computed TRANSPOSED (tiles are [j-part, i-free]) so
the Y normalization folds into the exp's per-partition scale and only the
2MB X operand needs a pre-scale.  fp8(e4m3) DoubleRow matmuls contract 256
k's per instruction at 0.5 cycles/row; the fp8 quantization noise on S
(~1e-3) is filtered by the 8192-term log-sums (out err ~1e-4) and the
diagonal terms are recomputed separately in bf16.

Sharding: X rows split across 8 cores; Y is fully replicated host-side so
each core streams all of Y from its own HBM at full DMA bandwidth instead
of over the collective links (the baseline's 28MB AllGather was 63% of its
critical path).  Each core:
 - computes its [8192 x 1024] block of S' = S.T: stationary = Y k-major
   per-j-tile, moving = its normalized X shard k-major,
 - gets ||y_j||^2 as the diagonal of Y'Y Gram tiles on PE, reusing the
   data matmuls' stationary weights (no square pass, no extra input),
 - exp on ACT with scale=1/||y_j|| (partition dim = j), accumulating
   column-sum partials for free; row sums via jt-paired DoubleRow
   ones-matmuls on fp8 E tiles into 2 persistent PSUM banks,
 - a single 32KB ReduceScatter sums the column partials across cores.
"""

import numpy as np
import ml_dtypes
from contextlib import ExitStack

import concourse.bass as bass
import concourse.bacc as bacc
import concourse.mybir as mybir
import concourse.tile as tile
from concourse.bass_utils import run_bass_kernel_spmd

FP32 = mybir.dt.float32
BF16 = mybir.dt.bfloat16
FP8 = mybir.dt.float8e4

BS = 8192      # batch (rows of X and Y)
H = 2048       # hidden
NCORES = 8
RPC = BS // NCORES   # rows per core = 1024
EPS = 1e-8

NJT = BS // 128      # 64 j-tiles (full Y rows)
KT = H // 128        # 16 k-planes
MT = RPC // 128      # 8 m-tiles (own rows)
NIH = RPC // 512     # 2 i-halves (psum bank width)

AF = mybir.ActivationFunctionType
ALU = mybir.AluOpType
DR = mybir.MatmulPerfMode.DoubleRow


def build():
    nc = bacc.Bacc("TRN2", target_bir_lowering=False, num_devices=NCORES)
    XT = nc.dram_tensor("XT", [H, RPC], FP8, kind="ExternalInput")
    XS = nc.dram_tensor("XS", [RPC, H], BF16, kind="ExternalInput")
    YD = nc.dram_tensor("YD", [RPC, H], BF16, kind="ExternalInput")
    YT = nc.dram_tensor("YT", [NJT, 128, KT, 128], FP8, kind="ExternalInput")
    EYE = nc.dram_tensor("EYE", [128, 128], FP32, kind="ExternalInput")
    OUT = nc.dram_tensor("OUT", [RPC, 1], FP32, kind="ExternalOutput")

    groups = [list(range(NCORES))]

    with tile.TileContext(nc) as tc, ExitStack() as ctx:
        dram = ctx.enter_context(tc.tile_pool(name="dram", bufs=1, space="DRAM"))
        stat = ctx.enter_context(tc.tile_pool(name="stat", bufs=1))
        wpool = ctx.enter_context(tc.tile_pool(name="wpool", bufs=1))
        rowp = ctx.enter_context(tc.tile_pool(name="rowp", bufs=3))
        junkp = ctx.enter_context(tc.tile_pool(name="junkp", bufs=3))
        ypool = ctx.enter_context(tc.tile_pool(name="ypool", bufs=4))
        epool = ctx.enter_context(tc.tile_pool(name="epool", bufs=4))
        small = ctx.enter_context(tc.tile_pool(name="small", bufs=4))
        psum = ctx.enter_context(tc.tile_pool(name="psum", bufs=4, space="PSUM"))
        cpsum = ctx.enter_context(tc.tile_pool(name="cpsum", bufs=1, space="PSUM"))
        gpsum = ctx.enter_context(tc.tile_pool(name="gpsum", bufs=2, space="PSUM"))

        CS = dram.tile([BS], FP32)
        CSR = dram.tile([RPC], FP32)
        RS1 = dram.tile([RPC], FP32)
        IXD = dram.tile([RPC], FP32)

        # persistent stats (partition p = row % 128, column m = row // 128)
        ssqx = stat.tile([128, MT], FP32)     # ||x_i||^2, own rows
        ssqyo = stat.tile([128, MT], FP32)    # ||y_i||^2, own rows
        sdraw = stat.tile([128, MT], FP32)    # x_i . y_i raw
        ssqy = stat.tile([128, NJT], FP32)    # ||y_j||^2, all rows (fp8 gram)
        invy_T = stat.tile([128, NJT], FP32)
        cacc = stat.tile([128, NJT, 2], FP32) # colsum partials per (jt, ih)
        eye = stat.tile([128, 128], FP32)
        ones8 = stat.tile([128, 1], FP8)
        ones_row = stat.tile([1, 128], BF16)

        nc.sync.dma_start(out=eye[:], in_=EYE[:, :])
        nc.vector.memset(ones8[:], 1.0)
        nc.vector.memset(ones_row[:], 1.0)

        # ---- raw X^T (fp8, k-major) ----
        xr = wpool.tile([128, KT, RPC], FP8)
        nc.sync.dma_start(out=xr[:], in_=XT.rearrange("(k p) m -> p k m", p=128))

        # ---- own-row stats: ||x||^2 (ACT), ||y||^2 and x.y (DVE) ----
        for m in range(MT):
            r0 = m * 128
            xs_m = rowp.tile([128, H], BF16, tag="xs", name="xs")
            nc.sync.dma_start(out=xs_m[:], in_=XS[r0 : r0 + 128, :])
            yd_m = rowp.tile([128, H], BF16, tag="yd", name="yd")
            nc.scalar.dma_start(out=yd_m[:], in_=YD[r0 : r0 + 128, :])
            junk = junkp.tile([128, H], BF16, tag="junk", name="junk")
            nc.scalar.activation(
                junk[:], xs_m[:], AF.Square, accum_out=ssqx[:, m : m + 1]
            )
            prod = junkp.tile([128, H], BF16, tag="junk", name="junk")
            nc.vector.tensor_mul(prod[:], yd_m[:], yd_m[:])
            nc.vector.reduce_sum(
                ssqyo[:, m : m + 1], prod[:], axis=mybir.AxisListType.X
            )
            prod2 = junkp.tile([128, H], BF16, tag="junk", name="junk")
            nc.vector.tensor_mul(prod2[:], xs_m[:], yd_m[:])
            nc.vector.reduce_sum(
                sdraw[:, m : m + 1], prod2[:], axis=mybir.AxisListType.X
            )

        def inv_chain(ssq_ap, out_ap, w, tag):
            """out = 1 / max(sqrt(ssq), eps)"""
            nrm = small.tile([128, w], FP32, tag=tag, name=tag)
            nc.scalar.sqrt(nrm[:], ssq_ap)
            nc.vector.tensor_scalar_max(nrm[:], nrm[:], EPS)
            nc.vector.reciprocal(out_ap, nrm[:])

        invx_pt = stat.tile([128, MT], FP32)
        invy_own = stat.tile([128, MT], FP32)
        inv_chain(ssqx[:], invx_pt[:], MT, "ix")
        inv_chain(ssqyo[:], invy_own[:], MT, "iy")

        # S_ii = (x_i.y_i) / (||x_i|| ||y_i||);  E_ii = exp(S_ii)
        sdiag = stat.tile([128, MT], FP32)
        nc.vector.tensor_mul(sdiag[:], sdraw[:], invx_pt[:])
        nc.vector.tensor_mul(sdiag[:], sdiag[:], invy_own[:])
        edig = stat.tile([128, MT], FP32)
        nc.scalar.activation(edig[:], sdiag[:], AF.Exp)

        # ---- broadcast invx along partitions: [128, MT] -> [128, 1024] ----
        # transpose via DRAM bounce, then MT K=1 ones-matmuls (into the data
        # psum pool, freed by rotation before the main loop needs 4 banks)
        nc.sync.dma_start(out=IXD.rearrange("(a b) -> b a", b=128), in_=invx_pt[:])
        ixf = small.tile([1, RPC], BF16, tag="ixf", name="ixf")
        nc.gpsimd.dma_start(out=ixf[:], in_=IXD.rearrange("a -> a"))
        invx_bc = wpool.tile([128, RPC], BF16)
        for h in range(NIH):
            bp = psum.tile([128, 512], FP32, tag="S", name="S")
            for q in range(4):
                g = h * 4 + q
                nc.tensor.matmul(
                    bp[:, q * 128 : (q + 1) * 128],
                    lhsT=ones_row[:], rhs=ixf[:, g * 128 : (g + 1) * 128],
                    start=True, stop=True,
                )
            nc.scalar.copy(invx_bc[:, h * 512 : (h + 1) * 512], bp[:])

        # xn = x * invx  (fp8, k-major)
        xn = wpool.tile([128, KT, RPC], FP8)
        for kt in range(KT):
            nc.vector.tensor_mul(xn[:, kt, :], xr[:, kt, :], invx_bc[:])

        # ---- main loop: S' tiles [j-part, i-free] ----
        rs_ps = [cpsum.tile([1, 512], FP32, tag=f"rs{h}", name=f"rs{h}")
                 for h in range(NIH)]
        NKK = KT // 2
        et = None
        for jt in range(NJT):
            yjt = ypool.tile([128, KT, 128], FP8, tag="yjt", name="yjt")
            eng = nc.sync if jt % 2 == 0 else nc.scalar
            eng.dma_start(out=yjt[:], in_=YT[jt])

            gram = gpsum.tile([128, 128], FP32, tag="g", name="g")
            pss = [psum.tile([128, 512], FP32, tag="S", name="S")
                   for _ in range(NIH)]
            for kk in range(NKK):
                w = yjt[:, 2 * kk : 2 * kk + 2, :]
                for ih in range(NIH):
                    nc.tensor.matmul(
                        pss[ih][:], lhsT=w,
                        rhs=xn[:, 2 * kk : 2 * kk + 2, ih * 512 : (ih + 1) * 512],
                        start=(kk == 0), stop=(kk == NKK - 1), perf_mode=DR,
                    )
                nc.tensor.matmul(
                    gram[:], lhsT=w, rhs=w,
                    start=(kk == 0), stop=(kk == NKK - 1), perf_mode=DR,
                )
            # ||y_j||^2 = diag(gram); 1/max(||y||,eps) per jt
            gd = junkp.tile([128, 128], FP32, tag="gd", name="gd")
            nc.vector.tensor_mul(gd[:], gram[:], eye[:])
            nc.vector.reduce_sum(
                ssqy[:, jt : jt + 1], gd[:], axis=mybir.AxisListType.X
            )
            nrm = small.tile([128, 1], FP32, tag="yn", name="yn")
            nc.scalar.sqrt(nrm[:], ssqy[:, jt : jt + 1])
            nc.vector.tensor_scalar_max(nrm[:], nrm[:], EPS)
            nc.vector.reciprocal(invy_T[:, jt : jt + 1], nrm[:])

            et = epool.tile([128, NIH, 512], FP8, tag="E", name="E")
            for ih in range(NIH):
                nc.scalar.activation(
                    et[:, ih, :], pss[ih][:], AF.Exp,
                    scale=invy_T[:, jt : jt + 1],
                    accum_out=cacc[:, jt, ih : ih + 1],
                )
                # row-sum partial: ones-matmul on fp8 E into persistent bank
                nc.tensor.matmul(
                    rs_ps[ih][:], lhsT=ones8[:], rhs=et[:, ih, :],
                    start=(jt == 0), stop=(jt == NJT - 1),
                )

        # ---- row sums -> [128, MT] via DRAM bounce ----
        rs_sb = small.tile([1, RPC], FP32, tag="rssb", name="rssb")
        for h in range(NIH):
            nc.vector.tensor_copy(rs_sb[:, h * 512 : (h + 1) * 512], rs_ps[h][:])
        nc.sync.dma_start(out=RS1.rearrange("a -> a"), in_=rs_sb[:])
        rstot = stat.tile([128, MT], FP32)
        nc.gpsimd.dma_start(out=rstot[:], in_=RS1.rearrange("(a b) -> b a", b=128))

        # ---- column sums: pairwise add, ReduceScatter ----
        csum = stat.tile([128, NJT], FP32)
        nc.vector.tensor_add(csum[:], cacc[:, :, 0], cacc[:, :, 1])
        nc.sync.dma_start(out=CS.rearrange("(a b) -> b a", b=128), in_=csum[:])
        nc.gpsimd.collective_compute(
            "ReduceScatter", ALU.add, replica_groups=groups,
            ins=[CS.opt()], outs=[CSR.opt()],
        )
        csr = stat.tile([128, MT], FP32)
        nc.gpsimd.dma_start(out=csr[:], in_=CSR.rearrange("(a b) -> b a", b=128))

        # ---- finale ----
        negr = stat.tile([128, MT], FP32)
        nc.vector.tensor_sub(negr[:], rstot[:], edig[:])
        negc = stat.tile([128, MT], FP32)
        nc.vector.tensor_sub(negc[:], csr[:], edig[:])
        lr = stat.tile([128, MT], FP32)
        nc.scalar.activation(lr[:], negr[:], AF.Ln)
        lc = stat.tile([128, MT], FP32)
        nc.scalar.activation(lc[:], negc[:], AF.Ln)
        res = stat.tile([128, MT], FP32)
        nc.vector.tensor_add(res[:], lr[:], lc[:])
        d2 = stat.tile([128, MT], FP32)
        nc.vector.tensor_scalar_mul(d2[:], sdiag[:], -2.0)
        nc.vector.tensor_add(res[:], res[:], d2[:])
        nc.sync.dma_start(
            out=OUT.rearrange("(a b) c -> b (a c)", b=128), in_=res[:]
        )

    nc.compile()
    return nc


_CACHE = {}


def _get_nc():
    if "nc" not in _CACHE:
        _CACHE["nc"] = build()
    return _CACHE["nc"]


def make_in_maps(X, Y, ncores=NCORES, rpc=RPC):
    fp8 = mybir.dt.np(FP8)
    bf16 = ml_dtypes.bfloat16
    X = np.asarray(X, np.float32)
    Y = np.asarray(Y, np.float32)
    # shared across all cores (replicated)
    YT4 = np.ascontiguousarray(
        Y.T.reshape(KT, 128, NJT, 128).transpose(2, 1, 0, 3)
    ).astype(fp8)
    EYEv = np.eye(128, dtype=np.float32)
    maps = []
    for i in range(ncores):
        xs = X[i * rpc : (i + 1) * rpc]
        maps.append({
            "XT": np.ascontiguousarray(xs.T).astype(fp8),
            "XS": xs.astype(bf16),
            "YD": Y[i * rpc : (i + 1) * rpc].astype(bf16),
            "YT": YT4,
            "EYE": EYEv,
        })
    return maps


def kernel(X, Y):
    X = np.ascontiguousarray(np.asarray(X, dtype=np.float32))
    Y = np.ascontiguousarray(np.asarray(Y, dtype=np.float32))
    assert X.shape == (BS, H) and Y.shape == (BS, H)
    nc = _get_nc()
    r = run_bass_kernel_spmd(nc, make_in_maps(X, Y), list(range(NCORES)))
    out = np.concatenate([r.results[i]["OUT"] for i in range(NCORES)], axis=0)
    return out.astype(np.float32)
